# revision 11
# baseline (speedup 1.0000x reference)
"""Trainium2 Bass kernel for nn_Ensemble (dense MLP ensemble, E=8, B=65536).

Network (per ensemble member e):
    x   = concat(inputs[..., :48], clip(inputs[..., 48:64], -1, 1))   # [B, 64]
    h1  = relu(x @ W1[e] + b1[e])                                     # [B, 128]
    h2  = relu(h1 @ W2[e] + b2[e])                                    # [B, 128]
    out = h2 @ W3[e] + b3[e]                                          # [B, 48]

Sharding: ensemble dim E=8 across the 8 NeuronCores (one member per core,
weights core-resident).  Feature-stationary layout: features on SBUF
partitions, batch on the free dim; all three layers are weight-stationary
matmuls streaming the batch.

  - Host packs x.T into X = [128, B/2] bf16: rows 0:64   = features, batch half 0
                                             rows 64:128 = features, batch half 1
    (clip folded into the host prep pass).
  - Unit = 512 X-cols (1024 samples).  Per unit:
      L1: 2 row-tiled concurrent MMs (PE rows 0:64 / 64:128) -> h1ps [128,1024]
      d1: ACT relu+bias drain -> h1sb bf16
      L2: 2 dense K=128 MMs -> h2ps [128,1024]
      d2: DVE relu+bias drain -> h2sb bf16
      L3: 2 col-tiled concurrent MMs (M=48 at col 0 / 64) -> ops [128,512]
      d3: bias drain split ACT [0:384] / DVE [384:512] -> o_t bf16
  - Software-pipelined issue order  L1(t) | L2(t-1) | L3(t-2)  keeps the PE
    FIFO free of drain stalls; ACT/DVE each carry ~1.45us/unit (balanced).
  - PSUM: h1ps bufs=1 (2 banks) + h2ps bufs=2 (4) + ops bufs=2 (2) = 8 banks.
  - Output DMA transfers only rows 0:48 / 64:112 (6 MB vs 8 MB).
"""

import numpy as np
import ml_dtypes

BF16 = ml_dtypes.bfloat16

E = 8
B = 65536
HB = B // 2          # batch half (X columns per core)
IN = 64
AC = 16              # clipped action features (last 16)
H = 128
OUT = 48

CHUNK = 8192         # X columns per x/out DMA chunk
U = 512              # X columns per pipeline unit (= 1024 samples)
D3A = 432            # columns of each ops tile drained by ACT (rest on DVE)

_CACHED = None


def _build_nc(reps=None):
    """Build the bass module. reps=None -> plain kernel; reps=R wraps the
    body in a hardware For_i loop (self-timing variant)."""
    import contextlib
    import concourse.bacc as bacc
    import concourse.mybir as mybir
    import concourse.tile as tile

    f32 = mybir.dt.float32
    bf16 = mybir.dt.bfloat16
    AF = mybir.ActivationFunctionType
    ALU = mybir.AluOpType

    nc = bacc.Bacc("TRN2", target_bir_lowering=False)

    x_d = nc.dram_tensor("x", [128, HB], bf16, kind="ExternalInput")
    w1_d = nc.dram_tensor("w1p", [128, H], bf16, kind="ExternalInput")
    w2_d = nc.dram_tensor("w2", [H, H], bf16, kind="ExternalInput")
    w3_d = nc.dram_tensor("w3", [H, OUT], bf16, kind="ExternalInput")
    b1_d = nc.dram_tensor("b1v", [H, 1], f32, kind="ExternalInput")
    b2_d = nc.dram_tensor("b2v", [H, 1], f32, kind="ExternalInput")
    b3_d = nc.dram_tensor("b3v", [128, 1], f32, kind="ExternalInput")
    out_d = nc.dram_tensor("out", [96, HB], bf16, kind="ExternalOutput")

    NCH = HB // CHUNK        # x/out chunks per core
    UPC = CHUNK // U         # units per chunk
    NU = HB // U             # total units
    LAG = 2                  # L3 trails L1 by 2 steps

    with tile.TileContext(nc) as tc:
        with (
            tc.tile_pool(name="consts", bufs=1) as consts,
            tc.tile_pool(name="xp", bufs=2) as xp,
            tc.tile_pool(name="h1p", bufs=3) as h1pool,
            tc.tile_pool(name="h2p", bufs=3) as h2pool,
            tc.tile_pool(name="osb", bufs=2) as opool,
            tc.tile_pool(name="ps1", bufs=2, space="PSUM") as ps1,
            tc.tile_pool(name="ps2", bufs=2, space="PSUM") as ps2,
            tc.tile_pool(name="ps3", bufs=2, space="PSUM") as ps3,
        ):
            w1_sb = consts.tile([128, H], bf16)
            w2_sb = consts.tile([H, H], bf16)
            w3_sb = consts.tile([H, OUT], bf16)
            b1_sb = consts.tile([H, 1], f32)
            b2_sb = consts.tile([H, 1], f32)
            b3_sb = consts.tile([128, 1], f32)
            nc.sync.dma_start(out=w1_sb, in_=w1_d[:])
            nc.sync.dma_start(out=w2_sb, in_=w2_d[:])
            nc.sync.dma_start(out=w3_sb, in_=w3_d[:])
            nc.sync.dma_start(out=b1_sb, in_=b1_d[:])
            nc.sync.dma_start(out=b2_sb, in_=b2_d[:])
            nc.sync.dma_start(out=b3_sb, in_=b3_d[:])

            loop = (tc.For_i(0, reps, 1, hint_engines=(mybir.EngineType.PE,))
                    if reps is not None else contextlib.nullcontext())
            with loop:
                xts = [None] * NCH     # SBUF x chunk tiles
                ots = [None] * NCH     # SBUF out chunk tiles
                h1sb = [None] * (NU + 1)
                h2sb = [None] * (NU + 1)
                opst = [None] * (NU + 1)

                for t in range(NU + LAG):
                    # -- x chunk DMA: issue chunk 0 (+1 ahead) at boundaries;
                    #    chunk 0 lands in fine slices so L1(0) starts early
                    if t < NU and t % UPC == 0:
                        c = t // UPC
                        todo = [0, 1] if c == 0 else ([c + 1] if c + 1 < NCH else [])
                        for cn in todo:
                            xt = xp.tile([128, CHUNK], bf16, name=f"xt{cn}",
                                         tag="xt")
                            nsl = 4 if cn == 0 else 2
                            sw = CHUNK // nsl
                            for s in range(nsl):
                                nc.sync.dma_start(
                                    out=xt[:, s * sw:(s + 1) * sw],
                                    in_=x_d[:, cn * CHUNK + s * sw:
                                            cn * CHUNK + (s + 1) * sw])
                            xts[cn] = xt

                    # -- stage 1: L1(t) + d1(t) [ACT]
                    if t < NU:
                        c, uc = t // UPC, (t % UPC) * U
                        x_t = xts[c]
                        h1ps = ps1.tile([128, 2 * U], f32, name="h1ps")
                        nc.tensor.matmul(h1ps[:, 0:U], w1_sb[0:64, :],
                                         x_t[0:64, uc:uc + U],
                                         start=True, stop=True)
                        nc.tensor.matmul(h1ps[:, U:2 * U], w1_sb[64:128, :],
                                         x_t[64:128, uc:uc + U],
                                         start=True, stop=True)
                        h1 = h1pool.tile([128, 2 * U], bf16, name="h1")
                        nc.scalar.activation(h1, h1ps, AF.Relu, bias=b1_sb)
                        h1sb[t] = h1

                    # -- stage 2: L2(t-1) + d2(t-1) [DVE]
                    if 1 <= t <= NU:
                        u = t - 1
                        h1 = h1sb[u]
                        h2psa = ps2.tile([128, U], f32, name="h2psa", tag="h2ps")
                        h2psb = ps2.tile([128, U], f32, name="h2psb", tag="h2ps")
                        h2 = h2pool.tile([128, 2 * U], bf16, name="h2")
                        nc.tensor.matmul(h2psa, w2_sb, h1[:, 0:U],
                                         start=True, stop=True)
                        nc.vector.tensor_scalar(h2[:, 0:U], h2psa, b2_sb, 0.0,
                                                op0=ALU.add, op1=ALU.max)
                        nc.tensor.matmul(h2psb, w2_sb, h1[:, U:2 * U],
                                         start=True, stop=True)
                        nc.vector.tensor_scalar(h2[:, U:2 * U], h2psb, b2_sb,
                                                0.0, op0=ALU.add, op1=ALU.max)
                        h2sb[u] = h2

                    # -- stage 3: L3(t-2) + d3(t-2) [ACT/DVE split]
                    if LAG <= t:
                        u = t - LAG
                        c, uc = u // UPC, (u % UPC) * U
                        if u % UPC == 0:
                            ots[c] = opool.tile([128, CHUNK], bf16,
                                                name=f"ot{c}", tag="ot")
                        h2 = h2sb[u]
                        ops = ps3.tile([128, U], f32, name="ops")
                        nc.tensor.matmul(ops[0:OUT, :], w3_sb, h2[:, 0:U],
                                         start=True, stop=True,
                                         tile_position=(0, 0))
                        nc.tensor.matmul(ops[64:64 + OUT, :], w3_sb,
                                         h2[:, U:2 * U],
                                         start=True, stop=True,
                                         tile_position=(0, 64))
                        o_t = ots[c]
                        nc.scalar.activation(o_t[:, uc:uc + D3A],
                                             ops[:, 0:D3A],
                                             AF.Identity, bias=b3_sb)
                        nc.vector.tensor_scalar(o_t[:, uc + D3A:uc + U],
                                                ops[:, D3A:U], b3_sb, None,
                                                op0=ALU.add)
                        # -- out DMA per 4 units (trims the pipeline tail)
                        if u % 4 == 3:
                            lo = (u % UPC - 3) * U
                            cs = slice(c * CHUNK + lo, c * CHUNK + lo + 4 * U)
                            nc.sync.dma_start(out=out_d[0:48, cs],
                                              in_=o_t[0:48, lo:lo + 4 * U])
                            nc.sync.dma_start(out=out_d[48:96, cs],
                                              in_=o_t[64:64 + OUT, lo:lo + 4 * U])

    nc.compile()
    return nc


def _get_nc():
    global _CACHED
    if _CACHED is None:
        _CACHED = _build_nc()
    return _CACHED


def _prep_member(x_e, W1_e, b1_e, W2_e, b2_e, W3_e, b3_e):
    """Host-side shard prep: transpose to feature-major, pack the two batch
    halves on the partition axis, clip action features, cast to bf16."""
    xt = np.ascontiguousarray(np.asarray(x_e).T)      # [64, B] f32
    np.clip(xt[IN - AC:IN], -1.0, 1.0, out=xt[IN - AC:IN])
    X = np.empty((128, HB), dtype=BF16)
    X[0:64] = xt[:, :HB]
    X[64:128] = xt[:, HB:]

    w1p = np.empty((128, H), dtype=BF16)
    w1p[0:64] = W1_e
    w1p[64:128] = W1_e
    w2 = np.asarray(W2_e, dtype=BF16)
    w3 = np.asarray(W3_e, dtype=BF16)
    b1v = np.ascontiguousarray(np.asarray(b1_e, np.float32).reshape(H, 1))
    b2v = np.ascontiguousarray(np.asarray(b2_e, np.float32).reshape(H, 1))
    b3v = np.zeros((128, 1), dtype=np.float32)
    b3v[0:OUT, 0] = b3_e
    b3v[64:64 + OUT, 0] = b3_e
    return {"x": X, "w1p": w1p, "w2": w2, "w3": w3,
            "b1v": b1v, "b2v": b2v, "b3v": b3v}


def kernel(**inputs):
    from concourse.bass_utils import run_bass_kernel_spmd

    x = np.asarray(inputs["inputs"], dtype=np.float32).reshape(E, B, IN)
    W1 = np.asarray(inputs["W1"], dtype=np.float32)
    b1 = np.asarray(inputs["b1"], dtype=np.float32)
    W2 = np.asarray(inputs["W2"], dtype=np.float32)
    b2 = np.asarray(inputs["b2"], dtype=np.float32)
    W3 = np.asarray(inputs["W3"], dtype=np.float32)
    b3 = np.asarray(inputs["b3"], dtype=np.float32)

    in_maps = [
        _prep_member(x[e], W1[e], b1[e], W2[e], b2[e], W3[e], b3[e])
        for e in range(E)
    ]

    nc = _get_nc()
    res = run_bass_kernel_spmd(nc, in_maps, core_ids=list(range(E)))

    out = np.empty((E, B, OUT), dtype=np.float32)
    for e in range(E):
        dev = res.results[e]["out"]          # [96, HB] bf16
        out[e, :HB] = dev[0:OUT, :].T
        out[e, HB:] = dev[OUT:2 * OUT, :].T
    return out


# revision 53
# speedup vs baseline: 10.4543x; 10.4543x over previous
"""Trainium2 Bass kernel for nn_Ensemble (dense MLP ensemble, E=8, B=65536).

Network (per ensemble member e):
    x   = concat(inputs[..., :48], clip(inputs[..., 48:64], -1, 1))   # [B, 64]
    h1  = relu(x @ W1[e] + b1[e])                                     # [B, 128]
    h2  = relu(h1 @ W2[e] + b2[e])                                    # [B, 128]
    out = h2 @ W3[e] + b3[e]                                          # [B, 48]

Sharding: ensemble dim E=8 across the 8 NeuronCores (one member per core,
weights core-resident).  Feature-stationary layout: features on SBUF
partitions, batch on the free dim; all three layers are weight-stationary
matmuls streaming the batch.

  - Host packs x.T into X = [128, B/2] bf16: rows 0:64   = features, batch half 0
                                             rows 64:128 = features, batch half 1
    (clip folded into the host prep pass).
  - Unit = 512 X-cols (1024 samples).  Per unit:
      L1: 2 row-tiled concurrent MMs (PE rows 0:64 / 64:128) -> h1ps [128,1024]
      d1: ACT relu+bias drain -> h1sb bf16
      L2: 2 dense K=128 MMs -> h2ps [128,1024]
      d2: DVE relu+bias drain -> h2sb bf16
      L3: 2 col-tiled concurrent MMs (M=48 at col 0 / 64) -> ops [128,512]
      d3: bias drain split ACT [0:384] / DVE [384:512] -> o_t bf16
  - Software-pipelined issue order  L1(t) | L2(t-1) | L3(t-2)  keeps the PE
    FIFO free of drain stalls; ACT/DVE each carry ~1.45us/unit (balanced).
  - PSUM: h1ps bufs=1 (2 banks) + h2ps bufs=2 (4) + ops bufs=2 (2) = 8 banks.
  - Output DMA transfers only rows 0:48 / 64:112 (6 MB vs 8 MB).
"""

import numpy as np
import ml_dtypes

BF16 = ml_dtypes.bfloat16

E = 8
B = 65536
HB = B // 2          # batch half (X columns per core)
IN = 64
AC = 16              # clipped action features (last 16)
H = 128
OUT = 48

CHUNK = 8192         # X columns per x/out DMA chunk
U = 512              # X columns per pipeline unit (= 1024 samples)
D3A = 432            # columns of each ops tile drained by ACT (rest on DVE)

_CACHED = None

# debug/bisect flags (leave off for grading)
_NO_XDMA = False     # skip x chunk DMAs (timing bisect only; wrong results)
_NO_ODMA = False     # skip out DMAs (timing bisect only)
_NO_DRAIN3 = False   # skip L3+d3 stage (timing bisect only)

# tuning knobs
_DRAIN_CFG = "min3"   # "split" | "whole3" | "fused2" | "min3" | "alt3"
_LAG = 2              # pipeline depth of stage 3 behind stage 1
_HSB_BUFS = 3         # h1sb/h2sb SBUF buffering
_D3A = None           # override for D3A split point
_TIMING_IO = False    # x/out as Internal DRAM (no host IO; timing only)
_ODMA_N = 4           # units per out DMA
_ODMA_GP = False      # out DMA via gpsimd (SWDGE) queue
_ORDER312 = False     # issue stage 3 before stages 1/2 within a step
_PE_PROBE = 0         # +1: extra L2 MM; -1: skip L3 MMs (timing probes)


def _build_nc(reps=None):
    """Build the bass module. reps=None -> plain kernel; reps=R wraps the
    body in a hardware For_i loop (self-timing variant)."""
    import contextlib
    import concourse.bacc as bacc
    import concourse.mybir as mybir
    import concourse.tile as tile

    f32 = mybir.dt.float32
    bf16 = mybir.dt.bfloat16
    AF = mybir.ActivationFunctionType
    ALU = mybir.AluOpType

    nc = bacc.Bacc("TRN2", target_bir_lowering=False)

    _ein = "Internal" if _TIMING_IO else "ExternalInput"
    _eout = "Internal" if _TIMING_IO else "ExternalOutput"
    x_d = nc.dram_tensor("x", [128, HB], bf16, kind=_ein)
    w1_d = nc.dram_tensor("w1p", [128, H], bf16, kind="ExternalInput")
    w2_d = nc.dram_tensor("w2", [H, H], bf16, kind="ExternalInput")
    w3_d = nc.dram_tensor("w3", [H, OUT], bf16, kind="ExternalInput")
    b1_d = nc.dram_tensor("b1v", [H, 1], f32, kind="ExternalInput")
    b2_d = nc.dram_tensor("b2v", [H, 1], f32, kind="ExternalInput")
    b3_d = nc.dram_tensor("b3v", [128, 1], f32, kind="ExternalInput")
    out_d = nc.dram_tensor("out", [128, HB], bf16, kind=_eout)
    probe_d = (nc.dram_tensor("probe", [128, 64], bf16, kind="ExternalOutput")
               if _TIMING_IO else None)

    D3 = _D3A if _D3A is not None else D3A
    NCH = HB // CHUNK        # x/out chunks per core
    UPC = CHUNK // U         # units per chunk
    NU = HB // U             # total units
    LAG = _LAG               # L3 trails L1 by this many steps

    with tile.TileContext(nc) as tc:
        with (
            tc.tile_pool(name="consts", bufs=1) as consts,
            tc.tile_pool(name="xp", bufs=2) as xp,
            tc.tile_pool(name="h1p", bufs=_HSB_BUFS) as h1pool,
            tc.tile_pool(name="h2p", bufs=_HSB_BUFS) as h2pool,
            tc.tile_pool(name="osb", bufs=2) as opool,
            tc.tile_pool(name="ps1",
                         bufs=(1 if _DRAIN_CFG in ("fused2", "min3", "alt3")
                               else 2),
                         space="PSUM") as ps1,
            tc.tile_pool(name="ps2", bufs=2, space="PSUM") as ps2,
            tc.tile_pool(name="ps3", bufs=2, space="PSUM") as ps3,
        ):
            w1_sb = consts.tile([128, H], bf16)
            w2_sb = consts.tile([H, H], bf16)
            w3_sb = consts.tile([H, OUT], bf16)
            b1_sb = consts.tile([H, 1], f32)
            b2_sb = consts.tile([H, 1], f32)
            b3_sb = consts.tile([128, 1], f32)
            nc.sync.dma_start(out=w1_sb, in_=w1_d[:])
            nc.sync.dma_start(out=w2_sb, in_=w2_d[:])
            nc.sync.dma_start(out=w3_sb, in_=w3_d[:])
            nc.sync.dma_start(out=b1_sb, in_=b1_d[:])
            nc.sync.dma_start(out=b2_sb, in_=b2_d[:])
            nc.sync.dma_start(out=b3_sb, in_=b3_d[:])

            loop = (tc.For_i(0, reps, 1, hint_engines=(mybir.EngineType.PE,))
                    if reps is not None else contextlib.nullcontext())
            with loop:
                xts = [None] * NCH     # SBUF x chunk tiles
                ots = [None] * NCH     # SBUF out chunk tiles
                h1sb = [None] * (NU + 1)
                h2sb = [None] * (NU + 1)
                opst = [None] * (NU + 1)

                def xdma(t):
                    # x chunk DMA: issue chunk 0 (+1 ahead) at boundaries;
                    # chunk 0 lands in fine slices so L1(0) starts early
                    if t < NU and t % UPC == 0:
                        c = t // UPC
                        todo = [0, 1] if c == 0 else ([c + 1] if c + 1 < NCH else [])
                        for cn in todo:
                            xt = xp.tile([128, CHUNK], bf16, name=f"xt{cn}",
                                         tag="xt")
                            nsl = 4 if cn == 0 else 2
                            if _NO_XDMA:  # keep one tiny write (bisect only)
                                nsl = 16
                            sw = CHUNK // nsl
                            for s in range(nsl if not _NO_XDMA else 1):
                                nc.sync.dma_start(
                                    out=xt[:, s * sw:(s + 1) * sw],
                                    in_=x_d[:, cn * CHUNK + s * sw:
                                            cn * CHUNK + (s + 1) * sw])
                            xts[cn] = xt

                def stage1(t):
                    # L1(t) + d1(t) [ACT]
                    if t < NU:
                        c, uc = t // UPC, (t % UPC) * U
                        x_t = xts[c]
                        h1ps = ps1.tile([128, 2 * U], f32, name="h1ps")
                        nc.tensor.matmul(h1ps[:, 0:U], w1_sb[0:64, :],
                                         x_t[0:64, uc:uc + U],
                                         start=True, stop=True)
                        nc.tensor.matmul(h1ps[:, U:2 * U], w1_sb[64:128, :],
                                         x_t[64:128, uc:uc + U],
                                         start=True, stop=True)
                        h1 = h1pool.tile([128, 2 * U], bf16, name="h1")
                        nc.scalar.activation(h1, h1ps, AF.Relu, bias=b1_sb)
                        h1sb[t] = h1

                def stage2(t):
                    # L2(t-1) + d2(t-1) [DVE]
                    if 1 <= t <= NU:
                        u = t - 1
                        h1 = h1sb[u]
                        h2 = h2pool.tile([128, 2 * U], bf16, name="h2")
                        if _DRAIN_CFG in ("fused2", "min3", "alt3"):
                            h2ps = ps2.tile([128, 2 * U], f32, name="h2ps",
                                            tag="h2ps")
                            nc.tensor.matmul(h2ps[:, 0:U], w2_sb, h1[:, 0:U],
                                             start=True, stop=True)
                            nc.tensor.matmul(h2ps[:, U:2 * U], w2_sb,
                                             h1[:, U:2 * U],
                                             start=True, stop=True)
                            if _PE_PROBE > 0:
                                nc.tensor.matmul(h2ps[:, U:2 * U], w2_sb,
                                                 h1[:, U:2 * U],
                                                 start=True, stop=True)
                            nc.vector.tensor_scalar(h2, h2ps, b2_sb, 0.0,
                                                    op0=ALU.add, op1=ALU.max)
                        else:
                            h2psa = ps2.tile([128, U], f32, name="h2psa",
                                             tag="h2ps")
                            h2psb = ps2.tile([128, U], f32, name="h2psb",
                                             tag="h2ps")
                            nc.tensor.matmul(h2psa, w2_sb, h1[:, 0:U],
                                             start=True, stop=True)
                            nc.vector.tensor_scalar(h2[:, 0:U], h2psa, b2_sb,
                                                    0.0, op0=ALU.add,
                                                    op1=ALU.max)
                            nc.tensor.matmul(h2psb, w2_sb, h1[:, U:2 * U],
                                             start=True, stop=True)
                            nc.vector.tensor_scalar(h2[:, U:2 * U], h2psb,
                                                    b2_sb, 0.0, op0=ALU.add,
                                                    op1=ALU.max)
                        h2sb[u] = h2

                def stage3(t):
                    # L3(t-2) + d3(t-2) [ACT/DVE split]
                    if LAG <= t and not _NO_DRAIN3:
                        u = t - LAG
                        c, uc = u // UPC, (u % UPC) * U
                        if u % UPC == 0:
                            ots[c] = opool.tile([128, CHUNK], bf16,
                                                name=f"ot{c}", tag="ot")
                        h2 = h2sb[u]
                        ops = ps3.tile([128, U], f32, name="ops")
                        if _PE_PROBE >= 0:
                            nc.tensor.matmul(ops[0:OUT, :], w3_sb, h2[:, 0:U],
                                             start=True, stop=True,
                                             tile_position=(0, 0))
                            nc.tensor.matmul(ops[64:64 + OUT, :], w3_sb,
                                             h2[:, U:2 * U],
                                             start=True, stop=True,
                                             tile_position=(0, 64))
                        else:  # timing probe: tiny write keeps ops allocated
                            nc.tensor.matmul(ops[0:OUT, 0:64], w3_sb,
                                             h2[:, 0:64],
                                             start=True, stop=True,
                                             tile_position=(0, 0))
                        o_t = ots[c]
                        if _DRAIN_CFG == "alt3":
                            # alternate d3's engine: ACT on even units, DVE
                            # on odd — balances ACT (d1+d3/2) vs DVE (d2+d3/2)
                            if u % 2 == 0:
                                nc.scalar.activation(o_t[:, uc:uc + U], ops,
                                                     AF.Identity, bias=b3_sb)
                            else:
                                nc.vector.tensor_scalar(o_t[:, uc:uc + U],
                                                        ops, b3_sb, None,
                                                        op0=ALU.add)
                        elif _DRAIN_CFG in ("whole3", "min3"):
                            nc.scalar.activation(o_t[:, uc:uc + U], ops,
                                                 AF.Identity, bias=b3_sb)
                        else:
                            nc.scalar.activation(o_t[:, uc:uc + D3],
                                                 ops[:, 0:D3],
                                                 AF.Identity, bias=b3_sb)
                            nc.vector.tensor_scalar(o_t[:, uc + D3:uc + U],
                                                    ops[:, D3:U], b3_sb, None,
                                                    op0=ALU.add)
                        # -- out DMA per _ODMA_N units (full 128 partitions:
                        #    rows 48:64 / 112:128 are pad, but partition-
                        #    sliced DMAs are far slower than the 2MB of waste)
                        if u % _ODMA_N == _ODMA_N - 1 and not _NO_ODMA:
                            lo = (u % UPC - (_ODMA_N - 1)) * U
                            w = _ODMA_N * U
                            cs = slice(c * CHUNK + lo, c * CHUNK + lo + w)
                            eng = nc.gpsimd if _ODMA_GP else nc.sync
                            eng.dma_start(out=out_d[:, cs],
                                          in_=o_t[:, lo:lo + w])

                for t in range(NU + LAG):
                    xdma(t)
                    if _ORDER312:
                        stage3(t)
                        stage1(t)
                        stage2(t)
                    else:
                        stage1(t)
                        stage2(t)
                        stage3(t)

            if probe_d is not None:
                # tiny sync output: forces full execution + blocking fetch
                pr = consts.tile([128, 64], bf16)
                nc.sync.dma_start(out=pr, in_=out_d[:, 0:64])
                nc.sync.dma_start(out=probe_d[:], in_=pr)

    nc.compile()
    return nc


def _get_nc():
    global _CACHED
    if _CACHED is None:
        _CACHED = _build_nc()
    return _CACHED


def _prep_member(x_e, W1_e, b1_e, W2_e, b2_e, W3_e, b3_e):
    """Host-side shard prep: transpose to feature-major, pack the two batch
    halves on the partition axis, clip action features, cast to bf16."""
    xt = np.ascontiguousarray(np.asarray(x_e).T)      # [64, B] f32
    np.clip(xt[IN - AC:IN], -1.0, 1.0, out=xt[IN - AC:IN])
    X = np.empty((128, HB), dtype=BF16)
    X[0:64] = xt[:, :HB]
    X[64:128] = xt[:, HB:]

    w1p = np.empty((128, H), dtype=BF16)
    w1p[0:64] = W1_e
    w1p[64:128] = W1_e
    w2 = np.asarray(W2_e, dtype=BF16)
    w3 = np.asarray(W3_e, dtype=BF16)
    b1v = np.ascontiguousarray(np.asarray(b1_e, np.float32).reshape(H, 1))
    b2v = np.ascontiguousarray(np.asarray(b2_e, np.float32).reshape(H, 1))
    b3v = np.zeros((128, 1), dtype=np.float32)
    b3v[0:OUT, 0] = b3_e
    b3v[64:64 + OUT, 0] = b3_e
    return {"x": X, "w1p": w1p, "w2": w2, "w3": w3,
            "b1v": b1v, "b2v": b2v, "b3v": b3v}


def kernel(**inputs):
    from concourse.bass_utils import run_bass_kernel_spmd

    x = np.asarray(inputs["inputs"], dtype=np.float32).reshape(E, B, IN)
    W1 = np.asarray(inputs["W1"], dtype=np.float32)
    b1 = np.asarray(inputs["b1"], dtype=np.float32)
    W2 = np.asarray(inputs["W2"], dtype=np.float32)
    b2 = np.asarray(inputs["b2"], dtype=np.float32)
    W3 = np.asarray(inputs["W3"], dtype=np.float32)
    b3 = np.asarray(inputs["b3"], dtype=np.float32)

    in_maps = [
        _prep_member(x[e], W1[e], b1[e], W2[e], b2[e], W3[e], b3[e])
        for e in range(E)
    ]

    nc = _get_nc()
    res = run_bass_kernel_spmd(nc, in_maps, core_ids=list(range(E)))

    out = np.empty((E, B, OUT), dtype=np.float32)
    for e in range(E):
        dev = res.results[e]["out"]          # [128, HB] bf16
        out[e, :HB] = dev[0:OUT, :].T
        out[e, HB:] = dev[64:64 + OUT, :].T
    return out
